# revision 32
# baseline (speedup 1.0000x reference)
"""Multi-head self-attention (1x1-conv projections, N=4096 spatial tokens,
C=256 channels, Cq=32) on 8 TRN2 NeuronCores, data-parallel over batch.

Per core (one batch element, x as [C, N]):
  q = wq @ x + bq          [Cq, N]
  k = wk @ x + bk          [Cq, N]
  v = wv @ x               [C, N]   (bv folded into the epilogue)
  S = q^T k                [N, N]
  P = softmax(S, axis=-1)
  out = gamma * (v @ P^T + bv) + x

Layout strategy: compute S^T tiles (keys j on partitions, queries i on the
free dim) so softmax's exp output E^T feeds the PV matmul as the stationary
operand with rhs = [v^T | ones]; the ones column accumulates the softmax
denominator for free (no P transposes, no separate reduction). exp skips
max-subtraction: S stays within +-45, far inside fp32 exp range; E in bf16.

Schedule: x streams in 256-col chunks; the q/k/v projections of chunk ch
interleave with i-block 0's attention units (energy quad -> exp -> PV) so
the PE never idles long enough for the HAM clock gate to re-throttle and
attention starts ~2us into the kernel instead of after a serial projection
prologue.  IB=256 with double-buffered 2-bank S^T PSUM tiles lets the four
32-row energy matmuls of a j-group issue concurrently (tile_position row
packing) with no write-after-read stall against the previous group's exp.

dtypes: fp32r (tf32-like) for the q/k/energy path where exp amplifies
absolute error; bf16 for the P*V path where softmax normalization cancels
it.  x is loaded once: projections read it through a fp32r bitcast view and
the residual-with-bias (x + gamma*bv) is added in place afterwards.
"""

import numpy as np

import concourse.bass as bass
import concourse.mybir as mybir
import concourse.tile as tile
from concourse.bass_utils import run_bass_kernel_spmd
from concourse.masks import make_identity
from concourse.tile import ScopedClock

F32 = mybir.dt.float32
F32R = mybir.dt.float32r
BF16 = mybir.dt.bfloat16

B, C, CQ = 8, 256, 32
H = W = 64
N = H * W            # 4096 tokens
NCORES = 8
CT = C // 128        # 2 channel tiles
IB = 256             # queries per i-block
N_IB = N // IB       # 16
JT = N // 128        # 32 key tiles
JGRP = 4             # key tiles per energy/exp group (one 2-bank PSUM tile)
N_JG = JT // JGRP    # 8
CH = 256             # x columns per load/projection chunk
N_CH = N // CH       # 16
SB = 512             # queries per energy super-block (2 PV i-blocks)


class PatchedTileContext(tile.TileContext):
    """This walrus build supports only ONE sync-wait command per
    instruction. Peel extra waits into standalone single-wait NOPs on the
    same engine queue, emitted immediately before the instruction (a serial
    conjunction of waits - semantically identical). Same treatment for the
    kernel-tail drain, whose global-clock waits otherwise all land on one
    Drain instruction."""

    MAX_WAITS_PER_INST = 1

    def _add_instruction(self, inst):
        si = inst.sync_info
        waits = list(si.on_wait) if si is not None and si.on_wait else []
        if len(waits) > self.MAX_WAITS_PER_INST and inst.engine is not None:
            keep = waits[-self.MAX_WAITS_PER_INST:]
            peel = waits[: -self.MAX_WAITS_PER_INST]
            for w in peel:
                nop = mybir.InstNoOp(
                    name=self.nc.get_next_instruction_name(),
                    ins=[],
                    outs=[],
                    sync_info=mybir.SyncInfo(on_wait=[w], on_update=[]),
                )
                nop.engine = inst.engine
                super()._add_instruction(nop)
            inst.sync_info = mybir.SyncInfo(
                on_wait=keep,
                on_update=list(si.on_update) if si.on_update else [],
            )
        super()._add_instruction(inst)

    def _drain_and_barrier(self, tick_clock, wait_clock):
        nc = self.nc
        carrier = nc.sync.nop()
        wait_clock.add_sem_waits(
            carrier.ins, ScopedClock({None: tick_clock.global_clock})
        )
        si = carrier.ins.sync_info
        waits = list(si.on_wait) if si is not None and si.on_wait else []
        carrier.ins.sync_info = None
        for w in waits:
            h = bass.SemaphoreHandle(name=w.ant_name or f"sem{w.id}", num=w.id)
            if w.wait_mode == "sem-ge-imm":
                nc.sync.wait_ge(h, w.wait_value)
            else:
                op = {
                    "sem-eq-imm": "eq",
                    "sem-le-imm": "le",
                    "sem-lt-imm": "lt",
                    "sem-gt-imm": "gt",
                }[w.wait_mode]
                nc.sync.wait_op(h, w.wait_value, op)
        nc.sync.drain()
        nc.all_engine_barrier()
        assert self.sems is not None
        popped = nc._tile_sem_poison_stack.pop()
        assert popped is self._sem_poison
        nc.clear_and_free_semaphores(list(self.sems.allocated().values()))
        nc.all_engine_barrier()


def _attention_body(nc, tc, ctx):
    x_e = nc.dram_tensor("x", [C, N], F32, kind="ExternalInput")
    wqt4_e = nc.dram_tensor("wqt4", [C, 128], F32, kind="ExternalInput")
    wkt4_e = nc.dram_tensor("wkt4", [C, 128], F32, kind="ExternalInput")
    wvt_e = nc.dram_tensor("wvt", [C, C], F32, kind="ExternalInput")
    bq4_e = nc.dram_tensor("bq4", [128, 1], F32, kind="ExternalInput")
    bk4_e = nc.dram_tensor("bk4", [128, 1], F32, kind="ExternalInput")
    bv_e = nc.dram_tensor("bv2", [128, CT], F32, kind="ExternalInput")
    gamma_e = nc.dram_tensor("gamma128", [128, 1], F32, kind="ExternalInput")
    out_e = nc.dram_tensor("out", [C, N], F32, kind="ExternalOutput")

    x_v = x_e.rearrange("(t p) n -> p t n", p=128)      # [128, CT, N]
    out_v = out_e.rearrange("(t p) n -> p t n", p=128)  # [128, CT, N]
    wqt_v = wqt4_e.rearrange("(t p) m -> p t m", p=128)
    wkt_v = wkt4_e.rearrange("(t p) m -> p t m", p=128)
    wvt_v = wvt_e.rearrange("(t p) m -> p t m", p=128)

    const = ctx.enter_context(tc.tile_pool(name="const", bufs=1))
    sb = ctx.enter_context(tc.tile_pool(name="sb", bufs=1))
    Ep = ctx.enter_context(tc.tile_pool(name="Ep", bufs=2))
    eps = ctx.enter_context(tc.tile_pool(name="eps", bufs=4))
    outp = ctx.enter_context(tc.tile_pool(name="outp", bufs=2))

    psS = ctx.enter_context(tc.tile_pool(name="psS", bufs=2, space="PSUM"))
    psO = ctx.enter_context(tc.tile_pool(name="psO", bufs=2, space="PSUM"))
    psM = ctx.enter_context(tc.tile_pool(name="psM", bufs=2, space="PSUM"))

    # ---- constants / weights (issued on scalar+gpsimd queues so their
    # descriptor-issue cost doesn't delay the x chunk DMAs on sync)
    bq4 = const.tile([128, 1], F32)
    bk4 = const.tile([128, 1], F32)
    bv2 = const.tile([128, CT], F32)
    gamma = const.tile([128, 1], F32)
    wq_f = const.tile([128, CT, 128], F32R)
    wk_f = const.tile([128, CT, 128], F32R)
    wv_f = const.tile([128, CT, C], F32R)
    nc.sync.dma_start(out=wq_f, in_=wqt_v.bitcast(F32R))
    nc.sync.dma_start(out=wk_f, in_=wkt_v.bitcast(F32R))
    nc.sync.dma_start(out=wv_f, in_=wvt_v.bitcast(F32R))
    nc.sync.dma_start(out=bq4, in_=bq4_e[:, :])
    nc.sync.dma_start(out=bk4, in_=bk4_e[:, :])
    nc.sync.dma_start(out=bv2, in_=bv_e[:, :])
    nc.sync.dma_start(out=gamma, in_=gamma_e[:, :])

    def wq_r(t):
        return wq_f[:, t, :]

    def wk_r(t):
        return wk_f[:, t, :]

    def wv_r(t):
        return wv_f[:, t, :]

    ident = const.tile([128, 128], BF16)
    make_identity(nc, ident)

    gbv = const.tile([128, CT], F32)
    nc.vector.tensor_scalar(
        out=gbv, in0=bv2, scalar1=gamma, scalar2=None, op0=mybir.AluOpType.mult
    )

    x_sb = sb.tile([128, CT, N], F32R)  # x (projection source, fp32r view)
    xb_sb = sb.tile([128, CT, N], F32)  # x + gamma*bv (residual for epilogue)
    qT = sb.tile([128, N], F32R)        # q^T replicated on 4 partition groups
    kT = sb.tile([128, N], F32R)
    v1T = sb.tile([128, JT, C + 1], BF16)  # [j-part, j-tile, c | ones]
    nc.vector.memset(v1T[:, :, C : C + 1], 1.0)

    def xr(t, s):
        return x_sb[:, t, s]

    def xf(t, s):
        return x_sb[:, t, s].bitcast(F32)

    E_of = {}

    def E_tile(sbk):
        # E for one super-block of SB=512 queries: [j-part, j-tile, i]
        if sbk not in E_of:
            E_of[sbk] = Ep.tile([128, JT, SB], BF16, tag="E", name=f"E_{sbk}")
        return E_of[sbk]

    po_of = {}

    def po_tiles(ib):
        if ib not in po_of:
            po_of[ib] = [
                psO.tile([128, C + 1], F32, tag="acc", name=f"po_{ib}_{i}")
                for i in range(IB // 128)
            ]
        return po_of[ib]

    def emit_energy(sbk, p):
        # S^T for j-tile pair p (jt 2p, 2p+1) x 512 queries of super-block
        # sbk, as one 2-bank PSUM tile.  Concurrently-draining row-packed
        # matmuls must target DISTINCT PSUM banks (one write port per
        # bank): the pair's two matmuls take the tile's two banks, and
        # consecutive pairs alternate PE row-group halves (p even: rows
        # 0-63, p odd: rows 64-127) so pair p+1 runs concurrently with
        # pair p into the other psS buffer's two banks - 4-way concurrency
        # across 4 distinct banks.  Pair p+2 reuses pair p's buffer but
        # the pool's WAR semaphore (exp(p) done) serializes it safely.
        isl = bass.ds(sbk * SB, SB)
        S = psS.tile([128, 2, SB], F32, tag="S", name=f"S_{sbk}_{p}")
        for g in range(2):
            jt = 2 * p + g
            gp = bass.ds(64 * (p % 2) + 32 * g, 32)
            nc.tensor.matmul(
                S[:, g, :],
                kT[gp, bass.ts(jt, 128)],
                qT[gp, isl],
                start=True, stop=True,
                tile_position=(64 * (p % 2) + 32 * g, 0),
            )
        nc.scalar.activation(
            out=E_tile(sbk)[:, 2 * p : 2 * p + 2, :],
            in_=S,
            func=mybir.ActivationFunctionType.Exp,
        )

    def emit_pv(ib, slot):
        # one slot = 4 j-tiles x both 128-query chunks of i-block ib
        po = po_tiles(ib)
        E = E_tile(ib // 2)
        ioff = (ib % 2) * IB
        for jt in range(4 * slot, 4 * slot + 4):
            for i_s in range(IB // 128):
                nc.tensor.matmul(
                    po[i_s],
                    E[:, jt, bass.ds(ioff + i_s * 128, 128)],
                    v1T[:, jt, :],
                    start=(jt == 0), stop=(jt == JT - 1),
                )

    def emit_xb(ch):
        # residual-with-bias in place, after every projection read of chunk
        sl = bass.ts(ch, CH)
        for t in range(CT):
            nc.vector.tensor_scalar(
                out=xb_sb[:, t, sl], in0=xf(t, sl),
                scalar1=gbv[:, t : t + 1], scalar2=None,
                op0=mybir.AluOpType.add,
            )

    def epilogue(ib):
        # normalize by the ones-column denominator, transpose to [c, n],
        # add residual, one batched store per i-block
        po = po_tiles(ib)
        ot = outp.tile([128, CT, IB], F32, tag="ot")
        for i_s in range(IB // 128):
            rd = eps.tile([128, 1], F32, tag="rd")
            nc.vector.reciprocal(out=rd, in_=po[i_s][:, C : C + 1])
            nc.vector.tensor_mul(out=rd, in0=rd, in1=gamma)
            pvn = eps.tile([128, C], BF16, tag="pvn")
            nc.vector.tensor_scalar(
                out=pvn, in0=po[i_s][:, 0:C], scalar1=rd, scalar2=None,
                op0=mybir.AluOpType.mult,
            )
            pt = psM.tile([128, 256], BF16, tag="pm", name=f"pt_{ib}_{i_s}")
            nc.tensor.transpose(pt[:, 0:128], pvn[:, 0:128], ident)
            nc.tensor.transpose(pt[:, 128:256], pvn[:, 128:256], ident)
            for t in range(CT):
                nc.vector.tensor_add(
                    out=ot[:, t, bass.ts(i_s, 128)],
                    in0=pt[:, bass.ts(t, 128)],
                    in1=xb_sb[:, t, bass.ds(ib * IB + i_s * 128, 128)],
                )
        for t in range(CT):
            nc.sync.dma_start(out=out_v[:, t, bass.ts(ib, IB)], in_=ot[:, t, :])

    # ---- prologue: x chunks + projections interleaved with i-block 0's
    # energy pairs and PV slots, so the PE never idles and attention
    # starts a couple of microseconds in
    for ch in range(N_CH):
        sl = bass.ts(ch, CH)
        nc.sync.dma_start(out=x_sb[:, :, sl], in_=x_v[:, :, sl].bitcast(F32R))
        pq = psM.tile([128, CH], F32, tag="pm", name=f"pq_{ch}")
        nc.tensor.matmul(pq, wq_r(0), xr(0, sl), start=True, stop=False)
        nc.tensor.matmul(pq, wq_r(1), xr(1, sl), start=False, stop=True)
        nc.vector.tensor_scalar(
            out=qT[:, sl], in0=pq, scalar1=bq4, scalar2=None,
            op0=mybir.AluOpType.add,
        )
        pk = psM.tile([128, CH], F32, tag="pm", name=f"pk_{ch}")
        nc.tensor.matmul(pk, wk_r(0), xr(0, sl), start=True, stop=False)
        nc.tensor.matmul(pk, wk_r(1), xr(1, sl), start=False, stop=True)
        nc.vector.tensor_scalar(
            out=kT[:, sl], in0=pk, scalar1=bk4, scalar2=None,
            op0=mybir.AluOpType.add,
        )
        for nt in range(2 * ch, 2 * ch + 2):
            pv = psM.tile([128, C], F32, tag="pm", name=f"pv_{nt}")
            nc.tensor.matmul(
                pv, xr(0, bass.ts(nt, 128)), wv_r(0), start=True, stop=False
            )
            nc.tensor.matmul(
                pv, xr(1, bass.ts(nt, 128)), wv_r(1), start=False, stop=True
            )
            nc.vector.tensor_copy(out=v1T[:, nt, 0:C], in_=pv)
        if ch >= 1:
            emit_energy(0, ch - 1)   # pair p needs kT chunk p + qT chunks 0-1
        if ch >= 4 and ch % 2 == 0:
            emit_pv(0, ch // 2 - 2)  # slots 0..5 at ch 4,6,..,14
    emit_energy(0, N_CH - 1)
    emit_pv(0, 6)
    emit_xb(0)
    emit_xb(1)
    emit_pv(0, 7)
    epilogue(0)

    # ---- steady state: while PV eats super-block I = ib//2, energy for
    # I+1 streams into the other E buffer (16 pairs spread over the two
    # i-blocks of I, except ib=1 which carries all 16 since ib=0's slots
    # were spent on super-block 0 inside the chunk loop)
    for ib in range(1, N_IB):
        nsb = ib // 2 + 1
        for slot in range(8):
            if nsb < N_IB // 2:
                if ib == 1:
                    emit_energy(nsb, 2 * slot)
                    emit_energy(nsb, 2 * slot + 1)
                elif ib % 2 == 0:
                    emit_energy(nsb, slot)
                else:
                    emit_energy(nsb, 8 + slot)
            emit_pv(ib, slot)
        if ib + 1 < N_CH:
            emit_xb(ib + 1)
        epilogue(ib)


_CACHE = {}


def _build():
    if "nc" not in _CACHE:
        nc = bass.Bass()
        from contextlib import ExitStack
        with PatchedTileContext(nc) as tc, ExitStack() as ctx:
            _attention_body(nc, tc, ctx)
        _CACHE["nc"] = nc
    return _CACHE["nc"]


def _prep_in_maps(x, wq, bq, wk, bk, wv, bv, gamma):
    asc = np.ascontiguousarray
    wqt4 = asc(np.tile(wq, (4, 1)).T.astype(np.float32))    # [C, 128]
    wkt4 = asc(np.tile(wk, (4, 1)).T.astype(np.float32))    # [C, 128]
    wvt = asc(wv.T.astype(np.float32))                      # [C, C]
    bq4 = asc(np.tile(bq, 4)[:, None].astype(np.float32))   # [128, 1]
    bk4 = asc(np.tile(bk, 4)[:, None].astype(np.float32))
    bv2 = asc(bv.reshape(CT, 128).T.astype(np.float32))     # [128, CT]
    g128 = np.full((128, 1), np.float32(gamma[0]), dtype=np.float32)
    maps = []
    for b in range(B):
        maps.append({
            "x": asc(x[b].reshape(C, N).astype(np.float32)),
            "wqt4": wqt4, "wkt4": wkt4, "wvt": wvt,
            "bq4": bq4, "bk4": bk4, "bv2": bv2, "gamma128": g128,
        })
    return maps


def _run(inputs, trace=False):
    nc = _build()
    in_maps = _prep_in_maps(**{k: np.asarray(v) for k, v in inputs.items()})
    res = run_bass_kernel_spmd(nc, in_maps, list(range(NCORES)), trace=trace)
    out = np.stack([res.results[b]["out"].reshape(C, H, W) for b in range(B)])
    return out.astype(np.float32), res


def kernel(**inputs):
    out, _ = _run(inputs, trace=False)
    return out
